# revision 7
# baseline (speedup 1.0000x reference)
"""Multi-head self-attention (B=4, S=2048, D=1024, H=16, causal) on 8 TRN2
NeuronCores, tensor-parallel over heads (2 heads per core).

Per-core computation (all matmuls in float32r — tf32-class, ~1e-4 rel):
  1. QKV projection for this core's 2 heads:  qT/kT/vT = W_c^T.T @ x^T,
     kept channel-major ([dk*2, seq]) in SBUF.
  2. v transposed to [seq, dk] tiles via PE transpose, augmented with a
     ones column (softmax denominator trick).
  3. Attention per (batch, q-block of 512): scores computed TRANSPOSED
     ([k, q] layout, lhsT = kT tile, rhs = qT block) so the probs feed the
     PV matmul directly with no transposes. exp on ScalarE (scale=1/8
     folded in, no max subtraction -- logits are O(+-4) here). Causal
     handling: fully-masked tiles skipped, diagonal tiles get a [128,128]
     triangle mask multiply. PV accumulates [65, q] = [v|1]^T @ expT into
     PSUM; row 64 is the softmax denominator. Normalization is applied to
     the [64, q] head output (32x cheaper than normalizing probs).
  4. Output projection partial: outT_c = (W_o[:, ch_c]^T).T @ headsT.
     Host sums the 8 partials and transposes back.
"""

import numpy as np

import concourse.bacc as bacc
import concourse.mybir as mybir
import concourse.tile as tile
from concourse import bass_utils

B, S, D, H, DK = 4, 2048, 1024, 16, 64
NCORES = 8
HPC = H // NCORES          # heads per core = 2
CH = HPC * DK              # qkv channels per head group = 128
NQ = 512                   # q block
KT = 128                   # k tile
NQB = S // NQ              # 4 q-blocks per batch
NKT = S // KT              # 16 k-tiles per batch
KD = D // 128              # 8 contraction tiles for projections

F32 = mybir.dt.float32
F32R = mybir.dt.float32r

_CACHE = {}


def _build():
    nc = bacc.Bacc("TRN2", target_bir_lowering=False, debug=False)

    xt_d = nc.dram_tensor("xt", [B, KD, 128, S], F32R, kind="ExternalInput")
    wqkvt_d = nc.dram_tensor("wqkvt", [KD, 128, 3 * CH], F32R, kind="ExternalInput")
    wot_d = nc.dram_tensor("wot", [CH, D], F32R, kind="ExternalInput")
    tri_d = nc.dram_tensor("tri", [128, 128], F32R, kind="ExternalInput")
    iden_d = nc.dram_tensor("iden", [128, 128], F32, kind="ExternalInput")
    outt_d = nc.dram_tensor("outt", [D // 128, 128, B * S], F32, kind="ExternalOutput")

    with tile.TileContext(nc) as tc:
        with (
            tc.tile_pool(name="xt", bufs=KD) as xt_pool,
            tc.tile_pool(name="qk", bufs=2) as qk_pool,
            tc.tile_pool(name="vst", bufs=2) as vst_pool,
            tc.tile_pool(name="vext", bufs=2 * NKT) as vext_pool,
            tc.tile_pool(name="expt", bufs=4) as expt_pool,
            tc.tile_pool(name="heads", bufs=2) as heads_pool,
            tc.tile_pool(name="wp", bufs=1) as w_pool,
            tc.tile_pool(name="outs", bufs=2) as out_pool,
            tc.tile_pool(name="small", bufs=4) as small_pool,
            tc.tile_pool(name="ppa", bufs=2, space="PSUM") as pp_a,
            tc.tile_pool(name="ppsc", bufs=3, space="PSUM") as pp_sc,
            tc.tile_pool(name="pppv", bufs=2, space="PSUM") as pp_pv,
            tc.tile_pool(name="ppt", bufs=1, space="PSUM") as pp_t,
        ):
            # --- persistent weights / constants ---
            wq = w_pool.tile([128, KD * 3 * CH], F32R, name="wq")
            for k in range(KD):
                nc.sync.dma_start(
                    out=wq[:, k * 3 * CH : (k + 1) * 3 * CH], in_=wqkvt_d.ap()[k]
                )
            wot = w_pool.tile([128, D], F32R, name="wot")
            nc.sync.dma_start(out=wot, in_=wot_d.ap())
            tri = w_pool.tile([128, 128], F32R, name="tri")
            nc.sync.dma_start(out=tri, in_=tri_d.ap())
            iden = w_pool.tile([128, 128], F32, name="iden")
            nc.sync.dma_start(out=iden, in_=iden_d.ap())
            ones32 = w_pool.tile([128, 1], F32, name="ones32")
            nc.vector.memset(ones32, 1.0)

            for b in range(B):
                # ---------- Phase A: QKV projection for batch b ----------
                xts = []
                for k in range(KD):
                    xtk = xt_pool.tile([128, S], F32R, name=f"xt_{b}_{k}", tag="xt")
                    nc.sync.dma_start(out=xtk, in_=xt_d.ap()[b, k])
                    xts.append(xtk)

                qT = qk_pool.tile([128, S], F32R, name=f"q_{b}", tag="q")
                kTt = qk_pool.tile([128, S], F32R, name=f"k_{b}", tag="k")
                vT = vst_pool.tile([128, S], F32, name=f"v_{b}", tag="v")
                dests = [qT, kTt, vT]
                for m in range(3):  # q, k, v channel groups
                    for n in range(S // 512):
                        ps = pp_a.tile([128, 512], F32, name=f"qkv_{b}_{m}_{n}", tag="mm")
                        for k in range(KD):
                            nc.tensor.matmul(
                                ps,
                                wq[:, k * 3 * CH + m * CH : k * 3 * CH + (m + 1) * CH],
                                xts[k][:, n * 512 : (n + 1) * 512],
                                start=(k == 0),
                                stop=(k == KD - 1),
                            )
                        # evacuate PSUM -> SBUF (ScalarE for q/k, DVE for v)
                        dst = dests[m][:, n * 512 : (n + 1) * 512]
                        if m < 2:
                            nc.scalar.copy(dst, ps)
                        else:
                            nc.vector.tensor_copy(out=dst, in_=ps)

                # ---------- Phase B: v -> [s, dk] tiles with ones column ----------
                vext = []
                for t in range(NKT):
                    pt = pp_t.tile([128, 128], F32, name=f"vt_{b}_{t}", tag="vt")
                    nc.tensor.transpose(pt, vT[:, t * 128 : (t + 1) * 128], iden)
                    ve = vext_pool.tile([128, 130], F32R, name=f"vext_{b}_{t}", tag="vext")
                    nc.scalar.copy(ve[:, 64:65], ones32)
                    nc.scalar.copy(ve[:, 129:130], ones32)
                    nc.scalar.copy(ve[:, 0:64], pt[:, 0:64])
                    nc.scalar.copy(ve[:, 65:129], pt[:, 64:128])
                    vext.append(ve)

                # ---------- Phase C: attention for batch b ----------
                headsT = heads_pool.tile([128, S], F32R, name=f"heads_{b}", tag="h")
                for qi in range(NQB):
                    pv0 = pp_pv.tile([128, 512], F32, name=f"pv0_{b}_{qi}", tag="pv")
                    pv1 = pp_pv.tile([128, 512], F32, name=f"pv1_{b}_{qi}", tag="pv")
                    jmax = (qi + 1) * (NQ // KT)  # k-tiles 0..jmax-1
                    for j in range(jmax):
                        diag = j * KT >= qi * NQ
                        n0 = j * KT - qi * NQ if diag else 0
                        q0 = qi * NQ
                        sc0 = pp_sc.tile([128, 512], F32, name=f"sc0_{b}_{qi}_{j}", tag="sc")
                        sc1 = pp_sc.tile([128, 512], F32, name=f"sc1_{b}_{qi}_{j}", tag="sc")
                        nc.tensor.matmul(
                            sc0[:, n0:512],
                            kTt[0:64, j * KT : (j + 1) * KT],
                            qT[0:64, q0 + n0 : q0 + 512],
                            start=True, stop=True,
                        )
                        nc.tensor.matmul(
                            sc1[:, n0:512],
                            kTt[64:128, j * KT : (j + 1) * KT],
                            qT[64:128, q0 + n0 : q0 + 512],
                            start=True, stop=True,
                        )
                        e0 = expt_pool.tile([128, 512], F32R, name=f"e0_{b}_{qi}_{j}", tag="e")
                        e1 = expt_pool.tile([128, 512], F32R, name=f"e1_{b}_{qi}_{j}", tag="e")
                        nc.scalar.activation(
                            e0[:, n0:512], sc0[:, n0:512],
                            mybir.ActivationFunctionType.Exp, scale=1.0 / np.sqrt(DK),
                        )
                        nc.scalar.activation(
                            e1[:, n0:512], sc1[:, n0:512],
                            mybir.ActivationFunctionType.Exp, scale=1.0 / np.sqrt(DK),
                        )
                        if diag:
                            nc.vector.tensor_mul(e0[:, n0 : n0 + 128], e0[:, n0 : n0 + 128], tri)
                            nc.vector.tensor_mul(e1[:, n0 : n0 + 128], e1[:, n0 : n0 + 128], tri)
                        last = j == jmax - 1
                        nc.tensor.matmul(
                            pv0[0:65, n0:512],
                            vext[j][:, 0:65],
                            e0[:, n0:512],
                            start=(j == 0), stop=last,
                        )
                        nc.tensor.matmul(
                            pv1[0:65, n0:512],
                            vext[j][:, 65:130],
                            e1[:, n0:512],
                            start=(j == 0), stop=last,
                        )
                    # epilogue: normalize by denominator row
                    for h, pv in ((0, pv0), (1, pv1)):
                        rc = small_pool.tile([1, 512], F32, name=f"rc{h}_{b}_{qi}", tag="rc")
                        nc.vector.reciprocal(rc, pv[64:65, :])
                        bc = small_pool.tile([64, 512], F32, name=f"bc{h}_{b}_{qi}", tag="bc")
                        nc.gpsimd.partition_broadcast(bc, rc, channels=64)
                        nc.vector.tensor_mul(
                            headsT[64 * h : 64 * h + 64, qi * NQ : (qi + 1) * NQ],
                            pv[0:64, :],
                            bc,
                        )

                # ---------- Phase D: output projection partial for batch b ----------
                for m in range(D // 128):
                    ot = out_pool.tile([128, S], F32, name=f"out_{b}_{m}", tag="o")
                    for n in range(S // 512):
                        ps = pp_a.tile([128, 512], F32, name=f"proj_{b}_{m}_{n}", tag="mm")
                        nc.tensor.matmul(
                            ps,
                            wot[:, m * 128 : (m + 1) * 128],
                            headsT[:, n * 512 : (n + 1) * 512],
                            start=True, stop=True,
                        )
                        nc.vector.tensor_copy(out=ot[:, n * 512 : (n + 1) * 512], in_=ps)
                    nc.sync.dma_start(out=outt_d.ap()[m, :, b * S : (b + 1) * S], in_=ot)

    nc.compile()
    return nc


def _get_nc():
    if "nc" not in _CACHE:
        _CACHE["nc"] = _build()
    return _CACHE["nc"]


def kernel(x, W_qkv, W_o):
    x = np.asarray(x, dtype=np.float32)
    W_qkv = np.asarray(W_qkv, dtype=np.float32)
    W_o = np.asarray(W_o, dtype=np.float32)

    xt = np.ascontiguousarray(x.transpose(0, 2, 1)).reshape(B, KD, 128, S)
    kk, qq = np.meshgrid(np.arange(128), np.arange(128), indexing="ij")
    tri = (kk <= qq).astype(np.float32)
    iden = np.eye(128, dtype=np.float32)

    in_maps = []
    for c in range(NCORES):
        r0 = CH * c
        rows = np.r_[r0 : r0 + CH, D + r0 : D + r0 + CH, 2 * D + r0 : 2 * D + r0 + CH]
        wqkvt = np.ascontiguousarray(W_qkv[rows].T).reshape(KD, 128, 3 * CH)
        wot = np.ascontiguousarray(W_o[:, r0 : r0 + CH].T)
        in_maps.append(
            {"xt": xt, "wqkvt": wqkvt, "wot": wot, "tri": tri, "iden": iden}
        )

    nc = _get_nc()
    res = bass_utils.run_bass_kernel_spmd(nc, in_maps, core_ids=list(range(NCORES)))
    _CACHE["last_result"] = res
    acc = np.zeros((D, B * S), dtype=np.float64)
    for c in range(NCORES):
        acc += res.results[c]["outt"].reshape(D, B * S).astype(np.float64)
    return np.ascontiguousarray(acc.T).reshape(B, S, D).astype(np.float32)


# revision 9
# speedup vs baseline: 1.3557x; 1.3557x over previous
"""Multi-head self-attention (B=4, S=2048, D=1024, H=16, causal) on 8 TRN2
NeuronCores, tensor-parallel over heads (2 heads per core).

Per-core computation (bf16 matmuls, fp32 PSUM accumulation):
  1. QKV projection for this core's 2 heads:  qT/kT/vT = W_c^T.T @ x^T,
     kept channel-major ([dk*2, seq]) in SBUF as bf16.
  2. v transposed to [seq, dk] tiles via PE transpose, augmented with a
     ones column (softmax denominator trick).
  3. Attention per (batch, q-block of 512): scores computed TRANSPOSED
     ([k, q] layout, lhsT = kT tile, rhs = qT block) so the probs feed the
     PV matmul directly with no transposes. exp on ScalarE (scale=1/8
     folded in, no max subtraction -- logits are O(+-4) here). Causal
     handling: fully-masked tiles skipped, diagonal tiles get a [128,128]
     triangle mask multiply. PV accumulates [65, q] = [v|1]^T @ expT into
     PSUM; row 64 is the softmax denominator. Normalization applies to the
     [64, q] head output (32x cheaper than normalizing probs).
  4. Output projection partial: outT_c = (W_o[:, ch_c]^T).T @ headsT.
     Host sums the 8 fp32 partials and transposes back.
"""

import numpy as np
import ml_dtypes

import concourse.bacc as bacc
import concourse.mybir as mybir
import concourse.tile as tile
from concourse import bass_utils

B, S, D, H, DK = 4, 2048, 1024, 16, 64
NCORES = 8
HPC = H // NCORES          # heads per core = 2
CH = HPC * DK              # qkv channels per head group = 128
NQ = 512                   # q block
KT = 128                   # k tile
NQB = S // NQ              # 4 q-blocks per batch
NKT = S // KT              # 16 k-tiles per batch
KD = D // 128              # 8 contraction tiles for projections

F32 = mybir.dt.float32
BF16 = mybir.dt.bfloat16

_CACHE = {}


def _build():
    nc = bacc.Bacc("TRN2", target_bir_lowering=False, debug=False)

    xt_d = nc.dram_tensor("xt", [B, KD, 128, S], BF16, kind="ExternalInput")
    wqkvt_d = nc.dram_tensor("wqkvt", [KD, 128, 3 * CH], BF16, kind="ExternalInput")
    wot_d = nc.dram_tensor("wot", [CH, D], BF16, kind="ExternalInput")
    tri_d = nc.dram_tensor("tri", [128, 128], BF16, kind="ExternalInput")
    iden_d = nc.dram_tensor("iden", [128, 128], BF16, kind="ExternalInput")
    outt_d = nc.dram_tensor("outt", [D // 128, 128, B * S], F32, kind="ExternalOutput")

    with tile.TileContext(nc) as tc:
        with (
            tc.tile_pool(name="xt", bufs=2 * KD) as xt_pool,
            tc.tile_pool(name="qk", bufs=2) as qk_pool,
            tc.tile_pool(name="vst", bufs=2) as vst_pool,
            tc.tile_pool(name="vext", bufs=2 * NKT) as vext_pool,
            tc.tile_pool(name="expt", bufs=6) as expt_pool,
            tc.tile_pool(name="heads", bufs=2) as heads_pool,
            tc.tile_pool(name="wp", bufs=1) as w_pool,
            tc.tile_pool(name="outs", bufs=3) as out_pool,
            tc.tile_pool(name="small", bufs=4) as small_pool,
            tc.tile_pool(name="ppa", bufs=2, space="PSUM") as pp_a,
            tc.tile_pool(name="ppsc", bufs=4, space="PSUM") as pp_sc,
            tc.tile_pool(name="pppv", bufs=2, space="PSUM") as pp_pv,
        ):
            # --- persistent weights / constants ---
            wq = w_pool.tile([128, KD * 3 * CH], BF16, name="wq")
            for k in range(KD):
                nc.sync.dma_start(
                    out=wq[:, k * 3 * CH : (k + 1) * 3 * CH], in_=wqkvt_d.ap()[k]
                )
            wot = w_pool.tile([128, D], BF16, name="wot")
            nc.sync.dma_start(out=wot, in_=wot_d.ap())
            tri = w_pool.tile([128, 128], BF16, name="tri")
            nc.sync.dma_start(out=tri, in_=tri_d.ap())
            iden = w_pool.tile([128, 128], BF16, name="iden")
            nc.sync.dma_start(out=iden, in_=iden_d.ap())
            ones32 = w_pool.tile([128, 1], F32, name="ones32")
            nc.vector.memset(ones32, 1.0)

            for b in range(B):
                # ---------- Phase A: QKV projection for batch b ----------
                xts = []
                for k in range(KD):
                    xtk = xt_pool.tile([128, S], BF16, name=f"xt_{b}_{k}", tag="xt")
                    nc.sync.dma_start(out=xtk, in_=xt_d.ap()[b, k])
                    xts.append(xtk)

                qT = qk_pool.tile([128, S], BF16, name=f"q_{b}", tag="q")
                kTt = qk_pool.tile([128, S], BF16, name=f"k_{b}", tag="k")
                vT = vst_pool.tile([128, S], BF16, name=f"v_{b}", tag="v")
                dests = [qT, kTt, vT]
                for m in range(3):  # q, k, v channel groups
                    for n in range(S // 512):
                        ps = pp_a.tile([128, 512], F32, name=f"qkv_{b}_{m}_{n}", tag="mm")
                        for k in range(KD):
                            nc.tensor.matmul(
                                ps,
                                wq[:, k * 3 * CH + m * CH : k * 3 * CH + (m + 1) * CH],
                                xts[k][:, n * 512 : (n + 1) * 512],
                                start=(k == 0),
                                stop=(k == KD - 1),
                            )
                        # evacuate PSUM -> SBUF bf16 (ScalarE for q/k, DVE for v)
                        dst = dests[m][:, n * 512 : (n + 1) * 512]
                        if m < 2:
                            nc.scalar.copy(dst, ps)
                        else:
                            nc.vector.tensor_copy(out=dst, in_=ps)

                # ---------- Phase B: v -> [s, dk] tiles with ones column ----------
                vext = []
                for t in range(NKT):
                    pt = pp_a.tile([128, 128], BF16, name=f"vt_{b}_{t}", tag="mm")
                    nc.tensor.transpose(pt, vT[:, t * 128 : (t + 1) * 128], iden)
                    ve = vext_pool.tile([128, 130], BF16, name=f"vext_{b}_{t}", tag="vext")
                    nc.scalar.copy(ve[:, 64:65], ones32)
                    nc.scalar.copy(ve[:, 129:130], ones32)
                    nc.scalar.copy(ve[:, 0:64], pt[:, 0:64])
                    nc.scalar.copy(ve[:, 65:129], pt[:, 64:128])
                    vext.append(ve)

                # ---------- Phase C: attention for batch b ----------
                headsT = heads_pool.tile([128, S], BF16, name=f"heads_{b}", tag="h")
                for qi in range(NQB):
                    pv0 = pp_pv.tile([128, 512], F32, name=f"pv0_{b}_{qi}", tag="pv")
                    pv1 = pp_pv.tile([128, 512], F32, name=f"pv1_{b}_{qi}", tag="pv")
                    jmax = (qi + 1) * (NQ // KT)  # k-tiles 0..jmax-1
                    for j in range(jmax):
                        diag = j * KT >= qi * NQ
                        n0 = j * KT - qi * NQ if diag else 0
                        q0 = qi * NQ
                        sc0 = pp_sc.tile([128, 512], F32, name=f"sc0_{b}_{qi}_{j}", tag="sc")
                        sc1 = pp_sc.tile([128, 512], F32, name=f"sc1_{b}_{qi}_{j}", tag="sc")
                        nc.tensor.matmul(
                            sc0[:, n0:512],
                            kTt[0:64, j * KT : (j + 1) * KT],
                            qT[0:64, q0 + n0 : q0 + 512],
                            start=True, stop=True,
                        )
                        nc.tensor.matmul(
                            sc1[:, n0:512],
                            kTt[64:128, j * KT : (j + 1) * KT],
                            qT[64:128, q0 + n0 : q0 + 512],
                            start=True, stop=True,
                        )
                        e0 = expt_pool.tile([128, 512], BF16, name=f"e0_{b}_{qi}_{j}", tag="e")
                        e1 = expt_pool.tile([128, 512], BF16, name=f"e1_{b}_{qi}_{j}", tag="e")
                        nc.scalar.activation(
                            e0[:, n0:512], sc0[:, n0:512],
                            mybir.ActivationFunctionType.Exp, scale=1.0 / np.sqrt(DK),
                        )
                        nc.scalar.activation(
                            e1[:, n0:512], sc1[:, n0:512],
                            mybir.ActivationFunctionType.Exp, scale=1.0 / np.sqrt(DK),
                        )
                        if diag:
                            nc.vector.tensor_mul(e0[:, n0 : n0 + 128], e0[:, n0 : n0 + 128], tri)
                            nc.vector.tensor_mul(e1[:, n0 : n0 + 128], e1[:, n0 : n0 + 128], tri)
                        last = j == jmax - 1
                        nc.tensor.matmul(
                            pv0[0:65, n0:512],
                            vext[j][:, 0:65],
                            e0[:, n0:512],
                            start=(j == 0), stop=last,
                        )
                        nc.tensor.matmul(
                            pv1[0:65, n0:512],
                            vext[j][:, 65:130],
                            e1[:, n0:512],
                            start=(j == 0), stop=last,
                        )
                    # epilogue: normalize by denominator row
                    for h, pv in ((0, pv0), (1, pv1)):
                        rc = small_pool.tile([1, 512], F32, name=f"rc{h}_{b}_{qi}", tag="rc")
                        nc.vector.reciprocal(rc, pv[64:65, :])
                        bc = small_pool.tile([64, 512], F32, name=f"bc{h}_{b}_{qi}", tag="bc")
                        nc.gpsimd.partition_broadcast(bc, rc, channels=64)
                        nc.vector.tensor_mul(
                            headsT[64 * h : 64 * h + 64, qi * NQ : (qi + 1) * NQ],
                            pv[0:64, :],
                            bc,
                        )

                # ---------- Phase D: output projection partial for batch b ----------
                for m in range(D // 128):
                    ot = out_pool.tile([128, S], F32, name=f"out_{b}_{m}", tag="o")
                    for n in range(S // 512):
                        ps = pp_a.tile([128, 512], F32, name=f"proj_{b}_{m}_{n}", tag="mm")
                        nc.tensor.matmul(
                            ps,
                            wot[:, m * 128 : (m + 1) * 128],
                            headsT[:, n * 512 : (n + 1) * 512],
                            start=True, stop=True,
                        )
                        if n % 2 == 0:
                            nc.vector.tensor_copy(out=ot[:, n * 512 : (n + 1) * 512], in_=ps)
                        else:
                            nc.scalar.copy(ot[:, n * 512 : (n + 1) * 512], ps)
                    nc.sync.dma_start(out=outt_d.ap()[m, :, b * S : (b + 1) * S], in_=ot)

    nc.compile()
    return nc


def _get_nc():
    if "nc" not in _CACHE:
        _CACHE["nc"] = _build()
    return _CACHE["nc"]


def kernel(x, W_qkv, W_o):
    x = np.asarray(x, dtype=np.float32)
    W_qkv = np.asarray(W_qkv, dtype=np.float32)
    W_o = np.asarray(W_o, dtype=np.float32)

    bf = ml_dtypes.bfloat16
    xt = np.ascontiguousarray(x.transpose(0, 2, 1)).reshape(B, KD, 128, S).astype(bf)
    kk, qq = np.meshgrid(np.arange(128), np.arange(128), indexing="ij")
    tri = (kk <= qq).astype(bf)
    iden = np.eye(128, dtype=bf)

    in_maps = []
    for c in range(NCORES):
        r0 = CH * c
        rows = np.r_[r0 : r0 + CH, D + r0 : D + r0 + CH, 2 * D + r0 : 2 * D + r0 + CH]
        wqkvt = np.ascontiguousarray(W_qkv[rows].T).reshape(KD, 128, 3 * CH).astype(bf)
        wot = np.ascontiguousarray(W_o[:, r0 : r0 + CH].T).astype(bf)
        in_maps.append(
            {"xt": xt, "wqkvt": wqkvt, "wot": wot, "tri": tri, "iden": iden}
        )

    nc = _get_nc()
    res = bass_utils.run_bass_kernel_spmd(nc, in_maps, core_ids=list(range(NCORES)))
    _CACHE["last_result"] = res
    acc = np.zeros((D, B * S), dtype=np.float64)
    for c in range(NCORES):
        acc += res.results[c]["outt"].reshape(D, B * S).astype(np.float64)
    return np.ascontiguousarray(acc.T).reshape(B, S, D).astype(np.float32)


# revision 10
# speedup vs baseline: 1.9496x; 1.4381x over previous
"""Multi-head self-attention (B=4, S=2048, D=1024, H=16, causal) on 8 TRN2
NeuronCores, tensor-parallel over heads (2 heads per core).

Per-core computation (bf16 matmuls, fp32 PSUM accumulation):
  1. QKV projection for this core's 2 heads kept channel-major in SBUF.
  2. v transposed to [seq, dk] tiles via PE transpose, + ones column
     (softmax denominator trick).
  3. Attention per (batch, q-block of 512): scores computed TRANSPOSED
     ([k, q] layout) so probs feed the PV matmul with no transposes.
     exp on ScalarE (scale=1/sqrt(dk) folded in, no max subtraction --
     logits are O(+-4) here). Causal: fully-masked tiles skipped,
     diagonal tiles get a [128,128] triangle mask multiply. PV
     accumulates [65, q] = [v|1]^T @ expT; row 64 = softmax denominator.
     Normalization applied to the [64, q] head output.
  4. Output projection partial: host sums the 8 fp32 partials.

The emission order software-pipelines batches: next batch's QKV/transpose
matmuls and previous batch's projection are interleaved into the attention
unit loop as PE filler, keeping TensorE dense (HAM stays at K=8/8).
"""

import numpy as np
import ml_dtypes

import concourse.bacc as bacc
import concourse.mybir as mybir
import concourse.tile as tile
from concourse import bass_utils

B, S, D, H, DK = 4, 2048, 1024, 16, 64
NCORES = 8
HPC = H // NCORES          # heads per core = 2
CH = HPC * DK              # qkv channels per head group = 128
NQ = 512                   # q block
KT = 128                   # k tile
NQB = S // NQ              # 4 q-blocks per batch
NKT = S // KT              # 16 k-tiles per batch
KD = D // 128              # 8 contraction tiles for projections

F32 = mybir.dt.float32
BF16 = mybir.dt.bfloat16

_CACHE = {}


def _build():
    nc = bacc.Bacc("TRN2", target_bir_lowering=False, debug=False)

    xt_d = nc.dram_tensor("xt", [B, KD, 128, S], BF16, kind="ExternalInput")
    wqkvt_d = nc.dram_tensor("wqkvt", [KD, 128, 3 * CH], BF16, kind="ExternalInput")
    wot_d = nc.dram_tensor("wot", [CH, D], BF16, kind="ExternalInput")
    tri_d = nc.dram_tensor("tri", [128, 128], BF16, kind="ExternalInput")
    iden_d = nc.dram_tensor("iden", [128, 128], BF16, kind="ExternalInput")
    outt_d = nc.dram_tensor("outt", [D // 128, 128, B * S], F32, kind="ExternalOutput")

    with tile.TileContext(nc) as tc:
        with (
            tc.tile_pool(name="xt", bufs=2 * KD) as xt_pool,
            tc.tile_pool(name="qk", bufs=2) as qk_pool,
            tc.tile_pool(name="vst", bufs=2) as vst_pool,
            tc.tile_pool(name="vext", bufs=2 * NKT) as vext_pool,
            tc.tile_pool(name="expt", bufs=6) as expt_pool,
            tc.tile_pool(name="heads", bufs=2) as heads_pool,
            tc.tile_pool(name="wp", bufs=1) as w_pool,
            tc.tile_pool(name="outs", bufs=3) as out_pool,
            tc.tile_pool(name="small", bufs=4) as small_pool,
            tc.tile_pool(name="ppa", bufs=2, space="PSUM") as pp_a,
            tc.tile_pool(name="ppsc", bufs=4, space="PSUM") as pp_sc,
            tc.tile_pool(name="pppv", bufs=2, space="PSUM") as pp_pv,
        ):
            # --- persistent weights / constants ---
            wq = w_pool.tile([128, KD * 3 * CH], BF16, name="wq")
            for k in range(KD):
                nc.sync.dma_start(
                    out=wq[:, k * 3 * CH : (k + 1) * 3 * CH], in_=wqkvt_d.ap()[k]
                )
            wot = w_pool.tile([128, D], BF16, name="wot")
            nc.sync.dma_start(out=wot, in_=wot_d.ap())
            tri = w_pool.tile([128, 128], BF16, name="tri")
            nc.sync.dma_start(out=tri, in_=tri_d.ap())
            iden = w_pool.tile([128, 128], BF16, name="iden")
            nc.sync.dma_start(out=iden, in_=iden_d.ap())
            ones32 = w_pool.tile([128, 1], F32, name="ones32")
            nc.vector.memset(ones32, 1.0)

            # per-batch state produced by phase A, consumed by C/D
            st = {}

            def gen_A(b):
                """QKV projection + v transposes for batch b. Yields ~20x."""
                xts = []
                for k in range(KD):
                    xtk = xt_pool.tile([128, S], BF16, name=f"xt_{b}_{k}", tag="xt")
                    nc.sync.dma_start(out=xtk, in_=xt_d.ap()[b, k])
                    xts.append(xtk)
                qT = qk_pool.tile([128, S], BF16, name=f"q_{b}", tag="q")
                kTt = qk_pool.tile([128, S], BF16, name=f"k_{b}", tag="k")
                vT = vst_pool.tile([128, S], BF16, name=f"v_{b}", tag="v")
                st[b] = {"q": qT, "k": kTt, "v": vT, "ve": []}
                dests = [qT, kTt, vT]
                for m in range(3):
                    for n in range(S // 512):
                        ps = pp_a.tile([128, 512], F32, name=f"qkv_{b}_{m}_{n}", tag="mm")
                        for k in range(KD):
                            nc.tensor.matmul(
                                ps,
                                wq[:, k * 3 * CH + m * CH : k * 3 * CH + (m + 1) * CH],
                                xts[k][:, n * 512 : (n + 1) * 512],
                                start=(k == 0),
                                stop=(k == KD - 1),
                            )
                        dst = dests[m][:, n * 512 : (n + 1) * 512]
                        if m < 2:
                            nc.scalar.copy(dst, ps)
                        else:
                            nc.vector.tensor_copy(out=dst, in_=ps)
                        yield
                for t in range(NKT):
                    pt = pp_a.tile([128, 128], BF16, name=f"vt_{b}_{t}", tag="mm")
                    nc.tensor.transpose(pt, vT[:, t * 128 : (t + 1) * 128], iden)
                    ve = vext_pool.tile([128, 130], BF16, name=f"vext_{b}_{t}", tag="vext")
                    nc.scalar.copy(ve[:, 64:65], ones32)
                    nc.scalar.copy(ve[:, 129:130], ones32)
                    nc.vector.tensor_copy(out=ve[:, 0:64], in_=pt[:, 0:64])
                    nc.vector.tensor_copy(out=ve[:, 65:129], in_=pt[:, 64:128])
                    st[b]["ve"].append(ve)
                    if t % 2 == 1:
                        yield

            def gen_D(b):
                """Output projection for batch b. Yields per m-tile (8x)."""
                headsT = st[b]["h"]
                for m in range(D // 128):
                    ot = out_pool.tile([128, S], F32, name=f"out_{b}_{m}", tag="o")
                    for n in range(S // 512):
                        ps = pp_a.tile([128, 512], F32, name=f"proj_{b}_{m}_{n}", tag="mm")
                        nc.tensor.matmul(
                            ps,
                            wot[:, m * 128 : (m + 1) * 128],
                            headsT[:, n * 512 : (n + 1) * 512],
                            start=True, stop=True,
                        )
                        if n % 2 == 0:
                            nc.vector.tensor_copy(out=ot[:, n * 512 : (n + 1) * 512], in_=ps)
                        else:
                            nc.scalar.copy(ot[:, n * 512 : (n + 1) * 512], ps)
                    nc.sync.dma_start(out=outt_d.ap()[m, :, b * S : (b + 1) * S], in_=ot)
                    yield

            def advance(fillers):
                for g in list(fillers):
                    try:
                        next(g)
                        return  # one chunk per call, round-robin via rotation
                    except StopIteration:
                        fillers.remove(g)

            def run_C(b, fillers):
                """Attention for batch b, with filler chunks interleaved."""
                qT, kTt = st[b]["q"], st[b]["k"]
                vext = st[b]["ve"]
                headsT = heads_pool.tile([128, S], BF16, name=f"heads_{b}", tag="h")
                st[b]["h"] = headsT
                for qi in range(NQB):
                    pv0 = pp_pv.tile([128, 512], F32, name=f"pv0_{b}_{qi}", tag="pv")
                    pv1 = pp_pv.tile([128, 512], F32, name=f"pv1_{b}_{qi}", tag="pv")
                    jmax = (qi + 1) * (NQ // KT)
                    for j in range(jmax):
                        diag = j * KT >= qi * NQ
                        n0 = j * KT - qi * NQ if diag else 0
                        q0 = qi * NQ
                        sc0 = pp_sc.tile([128, 512], F32, name=f"sc0_{b}_{qi}_{j}", tag="sc")
                        sc1 = pp_sc.tile([128, 512], F32, name=f"sc1_{b}_{qi}_{j}", tag="sc")
                        nc.tensor.matmul(
                            sc0[:, n0:512],
                            kTt[0:64, j * KT : (j + 1) * KT],
                            qT[0:64, q0 + n0 : q0 + 512],
                            start=True, stop=True,
                        )
                        nc.tensor.matmul(
                            sc1[:, n0:512],
                            kTt[64:128, j * KT : (j + 1) * KT],
                            qT[64:128, q0 + n0 : q0 + 512],
                            start=True, stop=True,
                        )
                        e0 = expt_pool.tile([128, 512], BF16, name=f"e0_{b}_{qi}_{j}", tag="e")
                        e1 = expt_pool.tile([128, 512], BF16, name=f"e1_{b}_{qi}_{j}", tag="e")
                        nc.scalar.activation(
                            e0[:, n0:512], sc0[:, n0:512],
                            mybir.ActivationFunctionType.Exp, scale=1.0 / np.sqrt(DK),
                        )
                        nc.scalar.activation(
                            e1[:, n0:512], sc1[:, n0:512],
                            mybir.ActivationFunctionType.Exp, scale=1.0 / np.sqrt(DK),
                        )
                        if diag:
                            nc.vector.tensor_mul(e0[:, n0 : n0 + 128], e0[:, n0 : n0 + 128], tri)
                            nc.vector.tensor_mul(e1[:, n0 : n0 + 128], e1[:, n0 : n0 + 128], tri)
                        last = j == jmax - 1
                        nc.tensor.matmul(
                            pv0[0:65, n0:512],
                            vext[j][:, 0:65],
                            e0[:, n0:512],
                            start=(j == 0), stop=last,
                        )
                        nc.tensor.matmul(
                            pv1[0:65, n0:512],
                            vext[j][:, 65:130],
                            e1[:, n0:512],
                            start=(j == 0), stop=last,
                        )
                        advance(fillers)
                    # normalize by denominator row
                    for h, pv in ((0, pv0), (1, pv1)):
                        dn = small_pool.tile([1, 512], F32, name=f"dn{h}_{b}_{qi}", tag="dn")
                        nc.scalar.copy(dn, pv[64:65, :])
                        rc = small_pool.tile([1, 512], F32, name=f"rc{h}_{b}_{qi}", tag="rc")
                        nc.vector.reciprocal_approx_fast(out=rc, in_=dn)
                        bc = small_pool.tile([64, 512], F32, name=f"bc{h}_{b}_{qi}", tag="bc")
                        nc.gpsimd.partition_broadcast(bc, rc, channels=64)
                        nc.vector.tensor_mul(
                            headsT[64 * h : 64 * h + 64, qi * NQ : (qi + 1) * NQ],
                            pv[0:64, :],
                            bc,
                        )

            # ---- software pipeline across batches ----
            for _ in gen_A(0):
                pass
            for b in range(B):
                fillers = []
                if b + 1 < B:
                    fillers.append(gen_A(b + 1))
                if b - 1 >= 0:
                    fillers.append(gen_D(b - 1))
                run_C(b, fillers)
                for g in fillers:
                    for _ in g:
                        pass
            for _ in gen_D(B - 1):
                pass

    nc.compile()
    return nc


def _get_nc():
    if "nc" not in _CACHE:
        _CACHE["nc"] = _build()
    return _CACHE["nc"]


def kernel(x, W_qkv, W_o):
    x = np.asarray(x, dtype=np.float32)
    W_qkv = np.asarray(W_qkv, dtype=np.float32)
    W_o = np.asarray(W_o, dtype=np.float32)

    bf = ml_dtypes.bfloat16
    xt = np.ascontiguousarray(x.transpose(0, 2, 1)).reshape(B, KD, 128, S).astype(bf)
    kk, qq = np.meshgrid(np.arange(128), np.arange(128), indexing="ij")
    tri = (kk <= qq).astype(bf)
    iden = np.eye(128, dtype=bf)

    in_maps = []
    for c in range(NCORES):
        r0 = CH * c
        rows = np.r_[r0 : r0 + CH, D + r0 : D + r0 + CH, 2 * D + r0 : 2 * D + r0 + CH]
        wqkvt = np.ascontiguousarray(W_qkv[rows].T).reshape(KD, 128, 3 * CH).astype(bf)
        wot = np.ascontiguousarray(W_o[:, r0 : r0 + CH].T).astype(bf)
        in_maps.append(
            {"xt": xt, "wqkvt": wqkvt, "wot": wot, "tri": tri, "iden": iden}
        )

    nc = _get_nc()
    res = bass_utils.run_bass_kernel_spmd(nc, in_maps, core_ids=list(range(NCORES)))
    _CACHE["last_result"] = res
    acc = np.zeros((D, B * S), dtype=np.float64)
    for c in range(NCORES):
        acc += res.results[c]["outt"].reshape(D, B * S).astype(np.float64)
    return np.ascontiguousarray(acc.T).reshape(B, S, D).astype(np.float32)
